# revision 28
# baseline (speedup 1.0000x reference)
"""Multi-head self-attention (softmax over query axis) on 8 TRN2 NeuronCores.

Sharding: core c -> (batch b = c // 4, head-group g = c % 4). Each head-group
owns 4 of the 16 heads (256 of the 1024 projected features). Each core computes
its batch's QKV projections for its 4 heads, the (column-softmax) attention,
and a partial output projection (row-parallel); the host sums the 4 bf16
partials per batch and adds the output bias.

v2 design (vs the dma-transpose baseline):
  - q/k/v are transposed to [D, S] on the HOST; the device does 3 large
    contiguous DMAs (no xbar-transpose DMAs, no PE transposes).
  - vh is produced directly in [s, e] layout (lhsT = xT s-slices).
  - causal mask is applied IN PSUM via one extra K=128 matmul per diagonal
    chunk: LT.T @ (-1e9*I) adds -1e9 wherever q <= k, so exp() gives exact
    zeros and the row sums from the exp's accum_out are already correct.
  - diagonal chunks skip their fully-masked column prefix entirely.
  - exp runs per (head, chunk) on the ACT engine with accum_out producing
    the softmax row-sums as a side effect (no DVE reduce over scores).
  - one software pipeline: ctx matmuls for kt trail the exp chain by
    PIPE_LAG kt steps so the PE never blocks the ACT stream; projection /
    out-projection matmuls are emitted as fillers between score units.
  - out partials are written as bf16 (half the DMA) and summed on host.
"""

import numpy as np
import ml_dtypes

import concourse.bass as bass
import concourse.mybir as mybir
import concourse.tile as tile
from concourse import bacc
from concourse.bass_utils import run_bass_kernel_spmd
from concourse.masks import make_identity

BF16 = ml_dtypes.bfloat16
B, S, D, H, HD = 2, 2048, 1024, 16, 64
NCORES = 8
GROUPS = 4          # head-groups (4 heads each)
EG = D // GROUPS    # 256 features per group
SCALE = 1.0 / np.sqrt(HD)  # 0.125
NEG = -1.0e9

bf = mybir.dt.bfloat16
f32 = mybir.dt.float32
EXP = mybir.ActivationFunctionType.Exp
IDENT = mybir.ActivationFunctionType.Identity


def _build_kernel(has_bias: bool):
    nc = bacc.Bacc(
        "TRN2",
        target_bir_lowering=False,
        debug=False,
        enable_asserts=False,
        num_devices=NCORES,
    )

    # ---- DRAM tensors (host pre-transposed / pre-packed, all bf16) ----
    # host-packed partition-major layouts: one contiguous run per partition
    xq_d = nc.dram_tensor("xq", [128, 8 * S], bf, kind="ExternalInput").ap()
    xk_d = nc.dram_tensor("xk", [128, 8 * S], bf, kind="ExternalInput").ap()
    xv_d = nc.dram_tensor("xv", [128, 8 * S], bf, kind="ExternalInput").ap()
    wq_d = nc.dram_tensor("wqt", [128, 8 * EG], bf, kind="ExternalInput").ap()
    wk_d = nc.dram_tensor("wkt", [128, 8 * EG], bf, kind="ExternalInput").ap()
    wv_d = nc.dram_tensor("wvt", [128, 8 * EG], bf, kind="ExternalInput").ap()
    wo_d = nc.dram_tensor("wot", [128, 2 * D], bf, kind="ExternalInput").ap()
    if has_bias:
        bq_d = nc.dram_tensor("bq_s", [EG], f32, kind="ExternalInput").ap()
        bk_d = nc.dram_tensor("bk_s", [EG], f32, kind="ExternalInput").ap()
        bvb_d = nc.dram_tensor("bv_b", [128, EG], bf, kind="ExternalInput").ap()
    out_d = nc.dram_tensor("out", [S, D], bf, kind="ExternalOutput").ap()

    with tile.TileContext(nc) as tc:
        with tc.tile_pool(name="persist", bufs=1) as P, \
             tc.tile_pool(name="xpool", bufs=1) as XP, \
             tc.tile_pool(name="epool", bufs=22) as EP, \
             tc.tile_pool(name="small", bufs=8) as SP, \
             tc.tile_pool(name="obuf", bufs=3) as OBP, \
             tc.tile_pool(name="scps", bufs=4, space="PSUM") as SCP, \
             tc.tile_pool(name="ctps", bufs=4, space="PSUM") as CTP:

            # ---------------- DMAs (issued in consumption order) ----------
            wq_t = P.tile([128, 8 * EG], bf, name="wq_t", tag="wq")
            nc.sync.dma_start(wq_t, wq_d)
            wk_t = P.tile([128, 8 * EG], bf, name="wk_t", tag="wk")
            nc.sync.dma_start(wk_t, wk_d)
            xq_t = XP.tile([128, 8 * S], bf, name="xq_t", tag="xq")
            nc.sync.dma_start(xq_t, xq_d)
            xk_t = XP.tile([128, 8 * S], bf, name="xk_t", tag="xk")
            nc.sync.dma_start(xk_t, xk_d)
            wv_t = P.tile([128, 8 * EG], bf, name="wv_t", tag="wv")
            xv_t = XP.tile([128, 8 * S], bf, name="xv_t", tag="xv")
            wo_t = P.tile([128, 2 * D], bf, name="wo_t", tag="wo")

            def issue_second_dma_batch():
                # WAW-gated: these transfers start only once xk has fully
                # landed (SDMA round-robins all active queues at packet
                # granularity, so issuing them earlier would slow xq/xk)
                for t in (wv_t, xv_t, wo_t):
                    nc.vector.memset(t[0:1, 0:1], 0.0)
                nc.sync.dma_start(wv_t, wv_d)
                nc.sync.dma_start(xv_t, xv_d)
                nc.sync.dma_start(wo_t, wo_d)

            bias_t = {}
            if has_bias:
                for nm, dram in (("bq", bq_d), ("bk", bk_d)):
                    for e in range(2):
                        t = P.tile([128, 1], f32, name=f"{nm}b{e}", tag=f"{nm}b{e}")
                        nc.sync.dma_start(
                            t, dram[e * 128:(e + 1) * 128].rearrange("(p o) -> p o", o=1)
                        )
                        bias_t[(nm, e)] = t
                bv_bt = P.tile([128, EG], bf, name="bv_bt", tag="bvb")
                nc.sync.dma_start(bv_bt, bvb_d)

            # ---------------- constants ----------------
            # dummy tile for HAM warm-up matmuls (DVE memset: ready ~1us in,
            # long before the gpsimd-built constants)
            wtile = P.tile([128, 128], bf, name="wtile", tag="wtile")
            nc.vector.memset(wtile, 0.0)
            # LT[i, k] = 1 where i <= k  (lower triangle of LT.T)
            LT = P.tile([128, 128], bf, name="LT", tag="LT")
            nc.gpsimd.memset(LT, 1.0)
            nc.gpsimd.affine_select(
                out=LT, in_=LT,
                compare_op=mybir.AluOpType.is_ge,
                fill=0.0, base=0, pattern=[[1, 128]], channel_multiplier=-1,
            )
            # negI = -1e9 * identity;  LT.T @ negI adds -1e9 where w <= k_local
            negI = P.tile([128, 128], bf, name="negI", tag="negI")
            make_identity(nc, negI)
            nc.vector.tensor_scalar_mul(negI, negI, NEG)
            ones_row = P.tile([1, 512], bf, name="ones_row", tag="ones_row")
            nc.gpsimd.memset(ones_row, 1.0)

            trash = P.tile([128, 512], bf, name="trash", tag="trash")

            # ---------------- persistent activations ----------------
            qhT = [P.tile([128, S], bf, name=f"qhT{e}", tag=f"qhT{e}") for e in range(2)]
            khT = [P.tile([128, S], bf, name=f"khT{e}", tag=f"khT{e}") for e in range(2)]
            vh = [P.tile([128, EG], bf, name=f"vh{st}", tag=f"vh{st}") for st in range(16)]
            ctxT = [P.tile([128, S], bf, name=f"ctxT{e}", tag=f"ctxT{e}") for e in range(2)]
            vspec = P.tile([1, EG], bf, name="vspec", tag="vspec")

            # ---------------- projection emitters ----------------
            open_proj = {}

            def emit_qk_half(x_t, w_t, dstT, e, sc, bias_key, half):
                key = (bias_key, e, sc)
                if half == 0:
                    ps = SCP.tile([128, 512], f32, name="pj", tag="sc")
                    open_proj[key] = ps
                else:
                    ps = open_proj.pop(key)
                for d in range(4 * half, 4 * half + 4):
                    nc.tensor.matmul(
                        ps,
                        lhsT=w_t[:, d * EG + e * 128: d * EG + (e + 1) * 128],
                        rhs=x_t[:, d * S + sc * 512: d * S + sc * 512 + 512],
                        start=(d == 0), stop=(d == 7),
                    )
                if half == 0:
                    return
                dst = dstT[e][:, sc * 512:(sc + 1) * 512]
                if has_bias:
                    nc.scalar.activation(
                        dst, ps, IDENT, bias=bias_t[(bias_key, e)], scale=1.0
                    )
                else:
                    nc.scalar.copy(dst, ps)

            def emit_qk_tile(x_t, w_t, dstT, e, sc, bias_key):
                ps = SCP.tile([128, 512], f32, name="pj", tag="sc")
                for d in range(8):
                    nc.tensor.matmul(
                        ps,
                        lhsT=w_t[:, d * EG + e * 128: d * EG + (e + 1) * 128],
                        rhs=x_t[:, d * S + sc * 512: d * S + sc * 512 + 512],
                        start=(d == 0), stop=(d == 7),
                    )
                dst = dstT[e][:, sc * 512:(sc + 1) * 512]
                if has_bias:
                    nc.scalar.activation(
                        dst, ps, IDENT, bias=bias_t[(bias_key, e)], scale=1.0
                    )
                elif e == 0:
                    nc.vector.tensor_copy(dst, ps)
                else:
                    nc.scalar.copy(dst, ps)

            def emit_v_tile(st):
                ps = SCP.tile([128, 512], f32, name="pv", tag="sc")
                for d in range(8):
                    nc.tensor.matmul(
                        ps[:, 0:EG],
                        lhsT=xv_t[:, d * S + st * 128: d * S + st * 128 + 128],
                        rhs=wv_t[:, d * EG:(d + 1) * EG],
                        start=(d == 0), stop=(d == 7),
                    )
                nc.vector.tensor_copy(vh[st], ps[:, 0:EG])
                if has_bias:
                    nc.vector.tensor_tensor(
                        out=vh[st], in0=vh[st], in1=bv_bt,
                        op=mybir.AluOpType.add,
                    )

            # ---- filler machinery: late projections are emitted between
            # attention units so the in-order PE queue never starves. ctx
            # matmuls for kt REQUIRE vh[kt]; ensure_v emits that projection
            # before any ctx consumer enters the PE queue.
            v_done = set()

            def ensure_v(st):
                if st not in v_done:
                    v_done.add(st)
                    emit_v_tile(st)

            def emit_filler(fillers):
                while fillers:
                    kind, e, idx, half = fillers.pop(0)
                    if kind == "q":
                        emit_qk_half(xq_t, wq_t, qhT, e, idx, "bq", half)
                    else:
                        emit_qk_half(xk_t, wk_t, khT, e, idx, "bk", half)
                    return True
                return False

            # ---------------- out-projection ----------------
            open_ob = {}

            def emit_outproj_half(st, oc):
                if oc == 0:
                    ob = OBP.tile([128, D], bf, name="ob", tag="ob")
                    open_ob[st] = ob
                else:
                    ob = open_ob.pop(st)
                ps = SCP.tile([128, 512], f32, name="op", tag="sc")
                for dk in range(2):
                    nc.tensor.matmul(
                        ps,
                        lhsT=ctxT[dk][:, st * 128:(st + 1) * 128],
                        rhs=wo_t[:, dk * D + oc * 512: dk * D + oc * 512 + 512],
                        start=(dk == 0), stop=(dk == 1),
                    )
                nc.vector.tensor_copy(ob[:, oc * 512:(oc + 1) * 512], ps)
                if oc == 1:
                    nc.sync.dma_start(out_d[st * 128:(st + 1) * 128, :], ob)

            def emit_outproj_st(st):
                emit_outproj_half(st, 0)
                emit_outproj_half(st, 1)

            # ---------------- attention ----------------
            def attention(p, fillers, outproj_sts, pipe_lag):
                # ctx psum tiles, one per 512-wide q block, accumulated over kt
                ctxp = [
                    CTP.tile([128, 512], f32, name=f"ctx{p}{qc}", tag="cop")
                    for qc in range(4)
                ]
                pend = []  # pipeline of (kt, units, rr)
                retired = set()

                def emit_ctx(kt, units, rr):
                    ensure_v(kt)  # vh[kt] producer must precede its consumers
                    # vh rows scaled by the softmax reciprocal; computed here
                    # (not at score time) so the DVE queue order is
                    # [vh copy, vhp scale] and never deadlocks
                    vhp = SP.tile([128, 128], bf, name="vhp", tag="vhp")
                    for a in range(2):
                        nc.vector.tensor_scalar_mul(
                            vhp[:, 64 * a:64 * (a + 1)],
                            vh[kt][:, 128 * p + 64 * a:128 * p + 64 * (a + 1)],
                            rr[:, a:a + 1],
                        )
                    for (a, qc, et, w, col0) in units:
                        nc.tensor.matmul(
                            ctxp[qc][64 * a:64 * (a + 1), col0:col0 + w],
                            lhsT=vhp[:, 64 * a:64 * (a + 1)],
                            rhs=et[:, 512 * a:512 * a + w],
                            start=(kt == 0), stop=False,
                            skip_group_check=True,
                        )

                def retire_qc(qc):
                    nc.tensor.matmul(
                        ctxp[qc],
                        lhsT=vspec[0:1, 128 * p:128 * (p + 1)],
                        rhs=ones_row,
                        start=False, stop=True,
                        skip_group_check=True,
                    )
                    nc.vector.tensor_copy(ctxT[p][:, qc * 512:(qc + 1) * 512], ctxp[qc])
                    retired.add(qc)

                def pop_stage(allow_retire):
                    okt, ounits, orr = pend.pop(0)
                    emit_ctx(okt, ounits, orr)
                    if allow_retire:
                        for qc_r in range(4):
                            if min(15, 4 * qc_r + 3) == okt:
                                retire_qc(qc_r)

                for kt in range(16):
                    qd, j = kt // 4, kt % 4
                    nch = 4 - qd
                    acc = SP.tile([128, 2, 4], f32, name="acc", tag="acc")
                    units = []
                    for ci, qc in enumerate(range(qd, 4)):
                        diag = qc == qd
                        w = 512 - 128 * j if diag else 512
                        qoff = qc * 512 + (128 * j if diag else 0)
                        col0 = qoff - qc * 512
                        # one E tile holds BOTH heads (h0 at 0, h1 at 512) so
                        # a single DVE reduce covers the pair's row sums
                        et = EP.tile([128, 1024], bf, name="E", tag="E")
                        use_accum = False  # tiny chunks: ACT accum is cheaper
                        for a in range(2):
                            ps = SCP.tile([128, 512], f32, name="scps", tag="sc")
                            nc.tensor.matmul(
                                ps[:, 0:w],
                                lhsT=khT[p][64 * a:64 * (a + 1), kt * 128:(kt + 1) * 128],
                                rhs=qhT[p][64 * a:64 * (a + 1), qoff:qoff + w],
                                start=True, stop=not diag,
                                skip_group_check=True,
                            )
                            if diag:
                                nc.tensor.matmul(
                                    ps[:, 0:128],
                                    lhsT=LT, rhs=negI,
                                    start=False, stop=True,
                                    skip_group_check=True,
                                )
                            nc.scalar.activation(
                                et[:, 512 * a:512 * a + w], ps[:, 0:w], EXP,
                                bias=0.0, scale=SCALE,
                                accum_out=(
                                    acc[:, a, ci:ci + 1] if use_accum else None
                                ),
                            )
                            units.append((a, qc, et, w, col0))
                        if not use_accum:
                            for a in range(2):
                                nc.vector.reduce_sum(
                                    acc[:, a, ci:ci + 1],
                                    et[:, 512 * a:512 * a + w],
                                    axis=mybir.AxisListType.X,
                                )
                        # between units: drain one pipeline stage / filler so
                        # the in-order PE queue always has runnable work.
                        # NOTE: v-projection tiles are emitted ONLY via the
                        # pipeline pops (ensure_v) — emitting them earlier
                        # would park the in-order PE queue on the xv DMA.
                        if ci == 0 and len(pend) >= pipe_lag:
                            pop_stage(allow_retire=(p == 1))
                        elif ci == 1:
                            emit_filler(fillers)
                        elif ci == 2:
                            if outproj_sts and outproj_sts[0][0] // 4 in retired:
                                emit_outproj_half(*outproj_sts.pop(0))
                            else:
                                emit_filler(fillers)

                    # short kts have few units: pump pops AND outproj work
                    if qd >= 2:
                        if len(pend) > 1:
                            pop_stage(allow_retire=(p == 1))
                        for _ in range(2):
                            if outproj_sts and outproj_sts[0][0] // 4 in retired:
                                emit_outproj_half(*outproj_sts.pop(0))

                    if p == 0 and kt == 12:
                        ensure_v(15)
                        nc.sync.dma_start(vspec, vh[15][127:128, :])
                        nc.gpsimd.tensor_scalar_mul(vspec, vspec, 1.0 / S)

                    # row sums -> reciprocal (vh scaling happens at pop time)
                    ssum = SP.tile([128, 2], f32, name="ssum", tag="ssum")
                    nc.vector.reduce_sum(
                        ssum, acc[:, :, 0:nch], axis=mybir.AxisListType.X
                    )
                    if kt == 15:
                        ssum2 = SP.tile([128, 2], f32, name="ssum2", tag="ssum2")
                        nc.vector.tensor_scalar_add(ssum2, ssum, 1.0e-30)
                        ssum = ssum2
                    rr = SP.tile([128, 2], f32, name="rr", tag="rr")
                    nc.vector.reciprocal(rr, ssum)
                    pend.append((kt, units, rr))

                # drain the pipeline
                while pend:
                    pop_stage(allow_retire=(p == 1))
                if p == 1:
                    for qc in range(4):
                        if qc not in retired:
                            retire_qc(qc)
                return retire_qc

            # ---------------- emission schedule ----------------
            # HAM warm-up: keep the PE busy while xq/xk stream in so the
            # projections run at 2.4 GHz instead of the cold 1.2 GHz
            warm = SCP.tile([128, 512], f32, name="warm", tag="sc")
            for _ in range(200):
                nc.tensor.matmul(
                    warm[:, 0:128], lhsT=wtile, rhs=wtile, start=True,
                    stop=True, skip_group_check=True,
                )
            # lead-in: q_e0 fully, then only k_sc0 (kt0-3 read k cols 0-511;
            # the rest arrives via fillers). A second warm-up burst bridges
            # the PE idle window between xq and xk arrival.
            for sc in range(4):
                emit_qk_tile(xq_t, wq_t, qhT, 0, sc, "bq")
            for _ in range(150):
                nc.tensor.matmul(
                    warm[:, 0:128], lhsT=wtile, rhs=wtile, start=True,
                    stop=True, skip_group_check=True,
                )
            emit_qk_tile(xk_t, wk_t, khT, 0, 0, "bk")
            issue_second_dma_batch()

            # fillers: the e1 halves of the q/k projections (xq/xk are
            # resident early; v tiles are pop-driven via ensure_v instead)
            fillers = []
            for sc in range(1, 4):
                for half in range(2):
                    fillers.append(("k", 0, sc, half))
            for sc in range(4):
                for half in range(2):
                    fillers.append(("q", 1, sc, half))
                for half in range(2):
                    fillers.append(("k", 1, sc, half))
            retire_p0 = attention(0, fillers, [], pipe_lag=4)
            # any projection fillers not consumed inside p0
            while emit_filler(fillers):
                pass

            for qc in range(4):
                retire_p0(qc)

            op_sts = [(st, oc) for st in range(16) for oc in range(2)]
            attention(1, [], op_sts, pipe_lag=1)  # pops op_sts in place
            for st, oc in op_sts:
                emit_outproj_half(st, oc)

    nc.compile()
    return nc


_NC_CACHE = {}


def _get_nc(has_bias: bool):
    if has_bias not in _NC_CACHE:
        _NC_CACHE[has_bias] = _build_kernel(has_bias)
    return _NC_CACHE[has_bias]


def _pack(a):
    # [n*128, m] -> [128, n*m] partition-major (one contiguous run/partition)
    n = a.shape[0] // 128
    return np.ascontiguousarray(
        a.reshape(n, 128, a.shape[1]).transpose(1, 0, 2).reshape(128, -1)
    )


def make_in_maps(q, k, v, Wq, bq, Wk, bk, Wv, bv, Wo, bo, has_bias):
    WqT = np.ascontiguousarray(Wq.T).astype(BF16)
    WkT = np.ascontiguousarray(Wk.T).astype(BF16)
    WvT = np.ascontiguousarray(Wv.T).astype(BF16)
    WoT = np.ascontiguousarray(Wo.T).astype(BF16)
    xqb = [_pack(q[b_].T.astype(BF16)) for b_ in range(B)]
    xkb = [_pack(k[b_].T.astype(BF16)) for b_ in range(B)]
    xvb = [_pack(v[b_].T.astype(BF16)) for b_ in range(B)]
    in_maps = []
    for c in range(NCORES):
        b_, g = c // GROUPS, c % GROUPS
        sl = slice(g * EG, (g + 1) * EG)
        m = {
            "xq": xqb[b_],
            "xk": xkb[b_],
            "xv": xvb[b_],
            "wqt": _pack(WqT[:, sl]),
            "wkt": _pack(WkT[:, sl]),
            "wvt": _pack(WvT[:, sl]),
            "wot": _pack(WoT[sl, :]),
        }
        if has_bias:
            m["bq_s"] = np.ascontiguousarray(bq[sl]).astype(np.float32)
            m["bk_s"] = np.ascontiguousarray(bk[sl]).astype(np.float32)
            m["bv_b"] = np.tile(
                np.ascontiguousarray(bv[sl]).astype(BF16)[None, :], (128, 1)
            )
        in_maps.append(m)
    return in_maps


def gather(results, bo):
    out = np.zeros((B, S, D), np.float32)
    for b_ in range(B):
        acc = np.zeros((S, D), np.float32)
        for g in range(GROUPS):
            acc += results[b_ * GROUPS + g]["out"].astype(np.float32)
        out[b_] = acc + bo.astype(np.float32)[None, :]
    return out


def kernel(q, k, v, Wq, bq, Wk, bk, Wv, bv, Wo, bo, **run_kwargs):
    q, k, v = (np.asarray(x, np.float32) for x in (q, k, v))
    Wq, bq, Wk, bk, Wv, bv, Wo, bo = (
        np.asarray(x, np.float32) for x in (Wq, bq, Wk, bk, Wv, bv, Wo, bo)
    )
    has_bias = bool(
        max(np.abs(bq).max(), np.abs(bk).max(), np.abs(bv).max()) > 0
    )
    nc = _get_nc(has_bias)
    in_maps = make_in_maps(q, k, v, Wq, bq, Wk, bk, Wv, bv, Wo, bo, has_bias)
    res = run_bass_kernel_spmd(
        nc, in_maps, core_ids=list(range(NCORES)), **run_kwargs
    )
    out = gather(res.results, bo)
    if run_kwargs:
        return out, res
    return out


# revision 30
# speedup vs baseline: 1.0062x; 1.0062x over previous
"""Multi-head self-attention (softmax over query axis) on 8 TRN2 NeuronCores.

Sharding: core c -> (batch b = c // 4, head-group g = c % 4). Each head-group
owns 4 of the 16 heads (256 of the 1024 projected features). Each core computes
its batch's QKV projections for its 4 heads, the (column-softmax) attention,
and a partial output projection (row-parallel); the host sums the 4 bf16
partials per batch and adds the output bias.

v2 design (vs the dma-transpose baseline):
  - q/k/v are transposed to [D, S] on the HOST; the device does 3 large
    contiguous DMAs (no xbar-transpose DMAs, no PE transposes).
  - vh is produced directly in [s, e] layout (lhsT = xT s-slices).
  - causal mask is applied IN PSUM via one extra K=128 matmul per diagonal
    chunk: LT.T @ (-1e9*I) adds -1e9 wherever q <= k, so exp() gives exact
    zeros and the row sums from the exp's accum_out are already correct.
  - diagonal chunks skip their fully-masked column prefix entirely.
  - exp runs per (head, chunk) on the ACT engine with accum_out producing
    the softmax row-sums as a side effect (no DVE reduce over scores).
  - one software pipeline: ctx matmuls for kt trail the exp chain by
    PIPE_LAG kt steps so the PE never blocks the ACT stream; projection /
    out-projection matmuls are emitted as fillers between score units.
  - out partials are written as bf16 (half the DMA) and summed on host.
"""

import numpy as np
import ml_dtypes

import concourse.bass as bass
import concourse.mybir as mybir
import concourse.tile as tile
from concourse import bacc
from concourse.bass_utils import run_bass_kernel_spmd
from concourse.masks import make_identity

BF16 = ml_dtypes.bfloat16
B, S, D, H, HD = 2, 2048, 1024, 16, 64
NCORES = 8
GROUPS = 4          # head-groups (4 heads each)
EG = D // GROUPS    # 256 features per group
SCALE = 1.0 / np.sqrt(HD)  # 0.125
NEG = -1.0e9

bf = mybir.dt.bfloat16
f32 = mybir.dt.float32
EXP = mybir.ActivationFunctionType.Exp
IDENT = mybir.ActivationFunctionType.Identity


def _build_kernel(has_bias: bool):
    nc = bacc.Bacc(
        "TRN2",
        target_bir_lowering=False,
        debug=False,
        enable_asserts=False,
        num_devices=NCORES,
    )

    # ---- DRAM tensors (host pre-transposed / pre-packed, all bf16) ----
    # host-packed partition-major layouts: one contiguous run per partition
    xq_d = nc.dram_tensor("xq", [128, 8 * S], bf, kind="ExternalInput").ap()
    xk_d = nc.dram_tensor("xk", [128, 8 * S], bf, kind="ExternalInput").ap()
    xv_d = nc.dram_tensor("xv", [128, 8 * S], bf, kind="ExternalInput").ap()
    wq_d = nc.dram_tensor("wqt", [128, 8 * EG], bf, kind="ExternalInput").ap()
    wk_d = nc.dram_tensor("wkt", [128, 8 * EG], bf, kind="ExternalInput").ap()
    wv_d = nc.dram_tensor("wvt", [128, 8 * EG], bf, kind="ExternalInput").ap()
    wo_d = nc.dram_tensor("wot", [128, 2 * D], bf, kind="ExternalInput").ap()
    if has_bias:
        bq_d = nc.dram_tensor("bq_s", [EG], f32, kind="ExternalInput").ap()
        bk_d = nc.dram_tensor("bk_s", [EG], f32, kind="ExternalInput").ap()
        bvb_d = nc.dram_tensor("bv_b", [128, EG], bf, kind="ExternalInput").ap()
    out_d = nc.dram_tensor("out", [S, D], bf, kind="ExternalOutput").ap()

    with tile.TileContext(nc) as tc:
        with tc.tile_pool(name="persist", bufs=1) as P, \
             tc.tile_pool(name="xpool", bufs=1) as XP, \
             tc.tile_pool(name="epool", bufs=22) as EP, \
             tc.tile_pool(name="small", bufs=8) as SP, \
             tc.tile_pool(name="obuf", bufs=3) as OBP, \
             tc.tile_pool(name="scps", bufs=4, space="PSUM") as SCP, \
             tc.tile_pool(name="ctps", bufs=4, space="PSUM") as CTP:

            # ---------------- DMAs (issued in consumption order) ----------
            wq_t = P.tile([128, 8 * EG], bf, name="wq_t", tag="wq")
            nc.sync.dma_start(wq_t, wq_d)
            wk_t = P.tile([128, 8 * EG], bf, name="wk_t", tag="wk")
            nc.sync.dma_start(wk_t, wk_d)
            xq_t = XP.tile([128, 8 * S], bf, name="xq_t", tag="xq")
            nc.sync.dma_start(xq_t, xq_d)
            xk_t = XP.tile([128, 8 * S], bf, name="xk_t", tag="xk")
            nc.sync.dma_start(xk_t, xk_d)
            wv_t = P.tile([128, 8 * EG], bf, name="wv_t", tag="wv")
            nc.sync.dma_start(wv_t, wv_d)
            xv_t = XP.tile([128, 8 * S], bf, name="xv_t", tag="xv")
            nc.sync.dma_start(xv_t, xv_d)
            wo_t = P.tile([128, 2 * D], bf, name="wo_t", tag="wo")
            nc.sync.dma_start(wo_t, wo_d)

            bias_t = {}
            if has_bias:
                for nm, dram in (("bq", bq_d), ("bk", bk_d)):
                    for e in range(2):
                        t = P.tile([128, 1], f32, name=f"{nm}b{e}", tag=f"{nm}b{e}")
                        nc.sync.dma_start(
                            t, dram[e * 128:(e + 1) * 128].rearrange("(p o) -> p o", o=1)
                        )
                        bias_t[(nm, e)] = t
                bv_bt = P.tile([128, EG], bf, name="bv_bt", tag="bvb")
                nc.sync.dma_start(bv_bt, bvb_d)

            # ---------------- constants ----------------
            # dummy tile for HAM warm-up matmuls (DVE memset: ready ~1us in,
            # long before the gpsimd-built constants)
            wtile = P.tile([128, 128], bf, name="wtile", tag="wtile")
            nc.vector.memset(wtile, 0.0)
            # LT[i, k] = 1 where i <= k  (lower triangle of LT.T)
            LT = P.tile([128, 128], bf, name="LT", tag="LT")
            nc.gpsimd.memset(LT, 1.0)
            nc.gpsimd.affine_select(
                out=LT, in_=LT,
                compare_op=mybir.AluOpType.is_ge,
                fill=0.0, base=0, pattern=[[1, 128]], channel_multiplier=-1,
            )
            # negI = -1e9 * identity;  LT.T @ negI adds -1e9 where w <= k_local
            negI = P.tile([128, 128], bf, name="negI", tag="negI")
            make_identity(nc, negI)
            nc.vector.tensor_scalar_mul(negI, negI, NEG)
            ones_row = P.tile([1, 512], bf, name="ones_row", tag="ones_row")
            nc.gpsimd.memset(ones_row, 1.0)

            trash = P.tile([128, 512], bf, name="trash", tag="trash")

            # ---------------- persistent activations ----------------
            qhT = [P.tile([128, S], bf, name=f"qhT{e}", tag=f"qhT{e}") for e in range(2)]
            khT = [P.tile([128, S], bf, name=f"khT{e}", tag=f"khT{e}") for e in range(2)]
            vh = [P.tile([128, EG], bf, name=f"vh{st}", tag=f"vh{st}") for st in range(16)]
            ctxT = [P.tile([128, S], bf, name=f"ctxT{e}", tag=f"ctxT{e}") for e in range(2)]
            vspec = P.tile([1, EG], bf, name="vspec", tag="vspec")

            # ---------------- projection emitters ----------------
            open_proj = {}

            def emit_qk_half(x_t, w_t, dstT, e, sc, bias_key, half):
                key = (bias_key, e, sc)
                if half == 0:
                    ps = SCP.tile([128, 512], f32, name="pj", tag="sc")
                    open_proj[key] = ps
                else:
                    ps = open_proj.pop(key)
                for d in range(4 * half, 4 * half + 4):
                    nc.tensor.matmul(
                        ps,
                        lhsT=w_t[:, d * EG + e * 128: d * EG + (e + 1) * 128],
                        rhs=x_t[:, d * S + sc * 512: d * S + sc * 512 + 512],
                        start=(d == 0), stop=(d == 7),
                    )
                if half == 0:
                    return
                dst = dstT[e][:, sc * 512:(sc + 1) * 512]
                if has_bias:
                    nc.scalar.activation(
                        dst, ps, IDENT, bias=bias_t[(bias_key, e)], scale=1.0
                    )
                elif bias_key == "bq":
                    nc.vector.tensor_copy(dst, ps)
                else:
                    nc.scalar.copy(dst, ps)

            def emit_qk_tile(x_t, w_t, dstT, e, sc, bias_key):
                ps = SCP.tile([128, 512], f32, name="pj", tag="sc")
                for d in range(8):
                    nc.tensor.matmul(
                        ps,
                        lhsT=w_t[:, d * EG + e * 128: d * EG + (e + 1) * 128],
                        rhs=x_t[:, d * S + sc * 512: d * S + sc * 512 + 512],
                        start=(d == 0), stop=(d == 7),
                    )
                dst = dstT[e][:, sc * 512:(sc + 1) * 512]
                if has_bias:
                    nc.scalar.activation(
                        dst, ps, IDENT, bias=bias_t[(bias_key, e)], scale=1.0
                    )
                elif e == 0:
                    nc.vector.tensor_copy(dst, ps)
                else:
                    nc.scalar.copy(dst, ps)

            def emit_v_tile(st):
                ps = SCP.tile([128, 512], f32, name="pv", tag="sc")
                for d in range(8):
                    nc.tensor.matmul(
                        ps[:, 0:EG],
                        lhsT=xv_t[:, d * S + st * 128: d * S + st * 128 + 128],
                        rhs=wv_t[:, d * EG:(d + 1) * EG],
                        start=(d == 0), stop=(d == 7),
                    )
                nc.vector.tensor_copy(vh[st], ps[:, 0:EG])
                if has_bias:
                    nc.vector.tensor_tensor(
                        out=vh[st], in0=vh[st], in1=bv_bt,
                        op=mybir.AluOpType.add,
                    )

            # ---- filler machinery: late projections are emitted between
            # attention units so the in-order PE queue never starves. ctx
            # matmuls for kt REQUIRE vh[kt]; ensure_v emits that projection
            # before any ctx consumer enters the PE queue.
            v_done = set()

            def ensure_v(st):
                if st not in v_done:
                    v_done.add(st)
                    emit_v_tile(st)

            def emit_filler(fillers):
                while fillers:
                    kind, e, idx, half = fillers.pop(0)
                    if kind == "v":
                        if idx in v_done:
                            continue
                        ensure_v(idx)
                    elif kind == "q":
                        emit_qk_half(xq_t, wq_t, qhT, e, idx, "bq", half)
                    else:
                        emit_qk_half(xk_t, wk_t, khT, e, idx, "bk", half)
                    return True
                return False

            # ---------------- out-projection ----------------
            open_ob = {}

            def emit_outproj_half(st, oc):
                if oc == 0:
                    ob = OBP.tile([128, D], bf, name="ob", tag="ob")
                    open_ob[st] = ob
                else:
                    ob = open_ob.pop(st)
                ps = SCP.tile([128, 512], f32, name="op", tag="sc")
                for dk in range(2):
                    nc.tensor.matmul(
                        ps,
                        lhsT=ctxT[dk][:, st * 128:(st + 1) * 128],
                        rhs=wo_t[:, dk * D + oc * 512: dk * D + oc * 512 + 512],
                        start=(dk == 0), stop=(dk == 1),
                    )
                nc.vector.tensor_copy(ob[:, oc * 512:(oc + 1) * 512], ps)
                if oc == 1:
                    nc.sync.dma_start(out_d[st * 128:(st + 1) * 128, :], ob)

            def emit_outproj_st(st):
                emit_outproj_half(st, 0)
                emit_outproj_half(st, 1)

            # ---------------- attention ----------------
            def attention(p, fillers, outproj_sts, pipe_lag):
                # ctx psum tiles, one per 512-wide q block, accumulated over kt
                ctxp = [
                    CTP.tile([128, 512], f32, name=f"ctx{p}{qc}", tag="cop")
                    for qc in range(4)
                ]
                pend = []  # pipeline of (kt, units, rr)
                retired = set()

                def emit_ctx(kt, units, rr):
                    ensure_v(kt)  # vh[kt] producer must precede its consumers
                    # vh rows scaled by the softmax reciprocal; computed here
                    # (not at score time) so the DVE queue order is
                    # [vh copy, vhp scale] and never deadlocks
                    vhp = SP.tile([128, 128], bf, name="vhp", tag="vhp")
                    for a in range(2):
                        nc.vector.tensor_scalar_mul(
                            vhp[:, 64 * a:64 * (a + 1)],
                            vh[kt][:, 128 * p + 64 * a:128 * p + 64 * (a + 1)],
                            rr[:, a:a + 1],
                        )
                    for (a, qc, et, w, col0) in units:
                        nc.tensor.matmul(
                            ctxp[qc][64 * a:64 * (a + 1), col0:col0 + w],
                            lhsT=vhp[:, 64 * a:64 * (a + 1)],
                            rhs=et[:, 512 * a:512 * a + w],
                            start=(kt == 0), stop=False,
                            skip_group_check=True,
                        )

                def retire_qc(qc):
                    nc.tensor.matmul(
                        ctxp[qc],
                        lhsT=vspec[0:1, 128 * p:128 * (p + 1)],
                        rhs=ones_row,
                        start=False, stop=True,
                        skip_group_check=True,
                    )
                    nc.vector.tensor_copy(ctxT[p][:, qc * 512:(qc + 1) * 512], ctxp[qc])
                    retired.add(qc)

                def pop_stage(allow_retire):
                    okt, ounits, orr = pend.pop(0)
                    emit_ctx(okt, ounits, orr)
                    if allow_retire:
                        for qc_r in range(4):
                            if min(15, 4 * qc_r + 3) == okt:
                                retire_qc(qc_r)

                for kt in range(16):
                    qd, j = kt // 4, kt % 4
                    nch = 4 - qd
                    acc = SP.tile([128, 2, 4], f32, name="acc", tag="acc")
                    units = []
                    for ci, qc in enumerate(range(qd, 4)):
                        diag = qc == qd
                        w = 512 - 128 * j if diag else 512
                        qoff = qc * 512 + (128 * j if diag else 0)
                        col0 = qoff - qc * 512
                        # one E tile holds BOTH heads (h0 at 0, h1 at 512) so
                        # a single DVE reduce covers the pair's row sums
                        et = EP.tile([128, 1024], bf, name="E", tag="E")
                        use_accum = False  # tiny chunks: ACT accum is cheaper
                        for a in range(2):
                            ps = SCP.tile([128, 512], f32, name="scps", tag="sc")
                            nc.tensor.matmul(
                                ps[:, 0:w],
                                lhsT=khT[p][64 * a:64 * (a + 1), kt * 128:(kt + 1) * 128],
                                rhs=qhT[p][64 * a:64 * (a + 1), qoff:qoff + w],
                                start=True, stop=not diag,
                                skip_group_check=True,
                            )
                            if diag:
                                nc.tensor.matmul(
                                    ps[:, 0:128],
                                    lhsT=LT, rhs=negI,
                                    start=False, stop=True,
                                    skip_group_check=True,
                                )
                            nc.scalar.activation(
                                et[:, 512 * a:512 * a + w], ps[:, 0:w], EXP,
                                bias=0.0, scale=SCALE,
                                accum_out=(
                                    acc[:, a, ci:ci + 1] if use_accum else None
                                ),
                            )
                            units.append((a, qc, et, w, col0))
                        if not use_accum:
                            for a in range(2):
                                nc.vector.reduce_sum(
                                    acc[:, a, ci:ci + 1],
                                    et[:, 512 * a:512 * a + w],
                                    axis=mybir.AxisListType.X,
                                )
                        # between units: drain one pipeline stage / filler so
                        # the in-order PE queue always has runnable work.
                        # NOTE: v-projection tiles are emitted ONLY via the
                        # pipeline pops (ensure_v) — emitting them earlier
                        # would park the in-order PE queue on the xv DMA.
                        if ci == 0 and len(pend) >= pipe_lag:
                            pop_stage(allow_retire=(p == 1))
                        elif ci == 1:
                            emit_filler(fillers)
                        elif ci == 2:
                            if outproj_sts and outproj_sts[0][0] // 4 in retired:
                                emit_outproj_half(*outproj_sts.pop(0))
                            else:
                                emit_filler(fillers)

                    # short kts have few units: pump pops AND outproj work
                    if qd >= 2:
                        if len(pend) > 1:
                            pop_stage(allow_retire=(p == 1))
                        for _ in range(2):
                            if outproj_sts and outproj_sts[0][0] // 4 in retired:
                                emit_outproj_half(*outproj_sts.pop(0))

                    if p == 0 and kt == 12:
                        ensure_v(15)
                        nc.sync.dma_start(vspec, vh[15][127:128, :])
                        nc.gpsimd.tensor_scalar_mul(vspec, vspec, 1.0 / S)

                    # row sums -> reciprocal (vh scaling happens at pop time)
                    ssum = SP.tile([128, 2], f32, name="ssum", tag="ssum")
                    nc.vector.reduce_sum(
                        ssum, acc[:, :, 0:nch], axis=mybir.AxisListType.X
                    )
                    if kt == 15:
                        ssum2 = SP.tile([128, 2], f32, name="ssum2", tag="ssum2")
                        nc.vector.tensor_scalar_add(ssum2, ssum, 1.0e-30)
                        ssum = ssum2
                    rr = SP.tile([128, 2], f32, name="rr", tag="rr")
                    nc.vector.reciprocal(rr, ssum)
                    pend.append((kt, units, rr))

                # drain the pipeline
                while pend:
                    pop_stage(allow_retire=(p == 1))
                if p == 1:
                    for qc in range(4):
                        if qc not in retired:
                            retire_qc(qc)
                return retire_qc

            # ---------------- emission schedule ----------------
            # HAM warm-up: keep the PE busy while xq/xk stream in so the
            # projections run at 2.4 GHz instead of the cold 1.2 GHz
            warm = SCP.tile([128, 512], f32, name="warm", tag="sc")
            for _ in range(250):
                nc.tensor.matmul(
                    warm[:, 0:128], lhsT=wtile, rhs=wtile, start=True,
                    stop=True, skip_group_check=True,
                )
            # lead-in: q_e0 fully, then only k_sc0 (kt0-3 read k cols 0-511;
            # the rest arrives via fillers). A second warm-up burst bridges
            # the PE idle window between xq and xk arrival.
            for sc in range(4):
                emit_qk_tile(xq_t, wq_t, qhT, 0, sc, "bq")
            for _ in range(110):
                nc.tensor.matmul(
                    warm[:, 0:128], lhsT=wtile, rhs=wtile, start=True,
                    stop=True, skip_group_check=True,
                )
            emit_qk_tile(xk_t, wk_t, khT, 0, 0, "bk")

            # fillers: the e1 halves of the q/k projections (xq/xk are
            # resident early; v tiles are pop-driven via ensure_v instead)
            fillers = []
            for sc in range(1, 4):
                for half in range(2):
                    fillers.append(("k", 0, sc, half))
            for sc in range(4):
                for half in range(2):
                    fillers.append(("q", 1, sc, half))
                for half in range(2):
                    fillers.append(("k", 1, sc, half))
                fillers.append(("v", 0, 2 * sc, 0))
                fillers.append(("v", 0, 2 * sc + 1, 0))
            for st in range(8, 10):
                fillers.append(("v", 0, st, 0))
            retire_p0 = attention(0, fillers, [], pipe_lag=4)
            # any projection fillers not consumed inside p0
            while emit_filler(fillers):
                pass

            for qc in range(4):
                retire_p0(qc)

            op_sts = [(st, oc) for st in range(16) for oc in range(2)]
            attention(1, [], op_sts, pipe_lag=1)  # pops op_sts in place
            for st, oc in op_sts:
                emit_outproj_half(st, oc)

    nc.compile()
    return nc


_NC_CACHE = {}


def _get_nc(has_bias: bool):
    if has_bias not in _NC_CACHE:
        _NC_CACHE[has_bias] = _build_kernel(has_bias)
    return _NC_CACHE[has_bias]


def _pack(a):
    # [n*128, m] -> [128, n*m] partition-major (one contiguous run/partition)
    n = a.shape[0] // 128
    return np.ascontiguousarray(
        a.reshape(n, 128, a.shape[1]).transpose(1, 0, 2).reshape(128, -1)
    )


def make_in_maps(q, k, v, Wq, bq, Wk, bk, Wv, bv, Wo, bo, has_bias):
    WqT = np.ascontiguousarray(Wq.T).astype(BF16)
    WkT = np.ascontiguousarray(Wk.T).astype(BF16)
    WvT = np.ascontiguousarray(Wv.T).astype(BF16)
    WoT = np.ascontiguousarray(Wo.T).astype(BF16)
    xqb = [_pack(q[b_].T.astype(BF16)) for b_ in range(B)]
    xkb = [_pack(k[b_].T.astype(BF16)) for b_ in range(B)]
    xvb = [_pack(v[b_].T.astype(BF16)) for b_ in range(B)]
    in_maps = []
    for c in range(NCORES):
        b_, g = c // GROUPS, c % GROUPS
        sl = slice(g * EG, (g + 1) * EG)
        m = {
            "xq": xqb[b_],
            "xk": xkb[b_],
            "xv": xvb[b_],
            "wqt": _pack(WqT[:, sl]),
            "wkt": _pack(WkT[:, sl]),
            "wvt": _pack(WvT[:, sl]),
            "wot": _pack(WoT[sl, :]),
        }
        if has_bias:
            m["bq_s"] = np.ascontiguousarray(bq[sl]).astype(np.float32)
            m["bk_s"] = np.ascontiguousarray(bk[sl]).astype(np.float32)
            m["bv_b"] = np.tile(
                np.ascontiguousarray(bv[sl]).astype(BF16)[None, :], (128, 1)
            )
        in_maps.append(m)
    return in_maps


def gather(results, bo):
    out = np.zeros((B, S, D), np.float32)
    for b_ in range(B):
        acc = np.zeros((S, D), np.float32)
        for g in range(GROUPS):
            acc += results[b_ * GROUPS + g]["out"].astype(np.float32)
        out[b_] = acc + bo.astype(np.float32)[None, :]
    return out


def kernel(q, k, v, Wq, bq, Wk, bk, Wv, bv, Wo, bo, **run_kwargs):
    q, k, v = (np.asarray(x, np.float32) for x in (q, k, v))
    Wq, bq, Wk, bk, Wv, bv, Wo, bo = (
        np.asarray(x, np.float32) for x in (Wq, bq, Wk, bk, Wv, bv, Wo, bo)
    )
    has_bias = bool(
        max(np.abs(bq).max(), np.abs(bk).max(), np.abs(bv).max()) > 0
    )
    nc = _get_nc(has_bias)
    in_maps = make_in_maps(q, k, v, Wq, bq, Wk, bk, Wv, bv, Wo, bo, has_bias)
    res = run_bass_kernel_spmd(
        nc, in_maps, core_ids=list(range(NCORES)), **run_kwargs
    )
    out = gather(res.results, bo)
    if run_kwargs:
        return out, res
    return out
